# revision 1
# baseline (speedup 1.0000x reference)
"""MAM fully-connected kernel for Trainium2, 8 NeuronCores (SPMD).

Computes out[m,n] = max_k(x[m,k]*w[n,k]) + min_k(x[m,k]*w[n,k]) + bias[n]
for x [8192,1024], weight [1024,1024], bias [1024] (all fp32 in/out).

Sharding: 8-way data-parallel over M; core c handles rows c*1024..+1024,
weight/bias replicated. Inputs are cast to fp16 host-side (the DVE computes
in fp32 internally; measured end-to-end error ~1.6e-3 scale-relative).

Device layout per core: partitions = k (8 tiles of 128), free = n.
Per m-group-of-8 (M_PER=8 rows share [128, 8192] working tiles):
  ScalarE: product tiles P_t = wt[t] * x[m,k-tile]   (activation Copy with
           per-partition scale)
  VectorE: running max / min chains over the 8 P_t   (tensor_tensor fp16 2x)
  TensorE: PE-transposes acc 128-chunks into PSUM    ([128, 16, 128] fp16)
  VectorE: tensor_reduce axis=X over k-partitions -> per-(m,n-block) results
Per 128-m group: D = C_max + C_min, PE-transpose back to [m,n], add bias,
fp32 staging -> DMA out.

The _split_waits post-pass works around this container's walrus build,
which rejects instructions carrying more than one semaphore wait.
"""

import sys

for _p in ("/opt/trn_rl_repo", "/root/.axon_site/_ro/trn_rl_repo"):
    if _p not in sys.path:
        sys.path.insert(0, _p)

import numpy as np

M, K, N = 8192, 1024, 1024
N_CORES = 8
MC = M // N_CORES  # 1024
P = 128
KT = K // P  # 8
MT = MC // P  # 8 output m-tiles / groups
M_PER = 8  # m's per DVE working tile
SPLIT_WAITS = True  # walrus needs <=1 wait/instruction (skip for CoreSim)
TS_SHIFT = 0  # product-halves moved from ScalarE to VectorE tensor_scalar per group

_compiled = {}
TRACE = False
LAST = {}


def _split_waits(nc, maxw=1):
    """Split >maxw sem-waits per instruction into preceding NoOps.

    The walrus build in this container rejects instructions whose sync_info
    carries more than one wait command ("Too many sync wait commands").
    Semantics are preserved: excess waits move onto NoOp instructions
    inserted immediately before the offender on the same engine queue.
    """
    import concourse.mybir as mybir

    uid = [0]
    for f in nc.m.functions:
        for bb in f.blocks:
            insts = bb.instructions
            out = []
            changed = False
            for inst in insts:
                si = inst.sync_info
                waits = list(si.on_wait) if (si and si.on_wait) else []
                if len(waits) > maxw:
                    chunks = [waits[i : i + maxw] for i in range(0, len(waits), maxw)]
                    for ch in chunks[:-1]:
                        uid[0] += 1
                        nop = mybir.InstNoOp(name=f"WS-{uid[0]}-{inst.name}")
                        nop.engine = inst.engine
                        nop.sync_info = mybir.SyncInfo(on_wait=ch, on_update=[])
                        out.append(nop)
                    inst.sync_info = mybir.SyncInfo(
                        on_wait=chunks[-1],
                        on_update=list(si.on_update) if si.on_update else [],
                    )
                    changed = True
                out.append(inst)
            if changed:
                bb.instructions = out




def _build():
    import concourse.bass as bass
    import concourse.tile as tile
    import concourse.mybir as mybir

    FP16 = mybir.dt.float16
    FP32 = mybir.dt.float32
    Alu = mybir.AluOpType
    Act = mybir.ActivationFunctionType
    W = M_PER * N  # working free width (4096)

    nc = bass.Bass()
    xt_d = nc.dram_tensor("xt", [KT, P, MC], FP32, kind="ExternalInput")
    wt_d = nc.dram_tensor("wt", [KT, P, N], FP16, kind="ExternalInput")
    brep_d = nc.dram_tensor("brep", [P, N], FP16, kind="ExternalInput")
    ident_d = nc.dram_tensor("ident", [P, P], FP16, kind="ExternalInput")
    out_d = nc.dram_tensor("out", [MT, P, N], FP32, kind="ExternalOutput")

    with tile.TileContext(nc) as tc:
        with (
            tc.tile_pool(name="res", bufs=1) as res,
            tc.tile_pool(name="prod", bufs=3) as prodp,
            tc.tile_pool(name="accp", bufs=2) as accp,
            tc.tile_pool(name="coll", bufs=2) as collp,
            tc.tile_pool(name="psA", bufs=2, space="PSUM") as psA,
            tc.tile_pool(name="psB", bufs=2, space="PSUM") as psB,
            tc.tile_pool(name="stg", bufs=2) as stgp,
        ):
            wt_t, xt_t = [], []
            for t in range(KT):
                w = res.tile([P, N], FP16, tag=f"w{t}")
                nc.sync.dma_start(w[:], wt_d[t])
                wt_t.append(w)
                xx = res.tile([P, MC], FP32, tag=f"x{t}")
                nc.sync.dma_start(xx[:], xt_d[t])
                xt_t.append(xx)
            brep = res.tile([P, N], FP16, tag="brep")
            nc.sync.dma_start(brep[:], brep_d[:])
            ident = res.tile([P, P], FP16, tag="ident")
            nc.sync.dma_start(ident[:], ident_d[:])

            uid = [0]

            def fresh(pool, shape, dt, tag):
                uid[0] += 1
                return pool.tile(shape, dt, name=f"{tag}_{uid[0]}", tag=tag)

            for g in range(MT):  # m-group of 128 rows
                c_max = fresh(collp, [P, P * KT], FP16, "cmax")
                c_min = fresh(collp, [P, P * KT], FP16, "cmin")
                for mp in range(P // M_PER):  # m-quads
                    m0 = g * P + mp * M_PER

                    def product(t):
                        pt = fresh(prodp, [P, W], FP16, "prod")
                        for h in range(M_PER):
                            if t == 0 and h < TS_SHIFT:
                                nc.vector.tensor_scalar(
                                    out=pt[:, h * N : (h + 1) * N],
                                    in0=wt_t[t][:],
                                    scalar1=xt_t[t][:, m0 + h : m0 + h + 1],
                                    scalar2=None,
                                    op0=Alu.mult,
                                )
                            else:
                                nc.scalar.activation(
                                    pt[:, h * N : (h + 1) * N],
                                    wt_t[t][:],
                                    Act.Copy,
                                    bias=0.0,
                                    scale=xt_t[t][:, m0 + h : m0 + h + 1],
                                )
                        return pt

                    # sequential max / min chains over the 8 k-tiles
                    p_prev = product(0)
                    p_cur = product(1)
                    a_max = fresh(accp, [P, W], FP16, "amax")
                    a_min = fresh(accp, [P, W], FP16, "amin")
                    nc.vector.tensor_tensor(
                        out=a_max[:], in0=p_prev[:], in1=p_cur[:], op=Alu.max
                    )
                    nc.vector.tensor_tensor(
                        out=a_min[:], in0=p_prev[:], in1=p_cur[:], op=Alu.min
                    )
                    for t in range(2, KT):
                        p_cur = product(t)
                        nxt_max = fresh(accp, [P, W], FP16, "amax")
                        nxt_min = fresh(accp, [P, W], FP16, "amin")
                        nc.vector.tensor_tensor(
                            out=nxt_max[:], in0=a_max[:], in1=p_cur[:], op=Alu.max
                        )
                        nc.vector.tensor_tensor(
                            out=nxt_min[:], in0=a_min[:], in1=p_cur[:], op=Alu.min
                        )
                        a_max, a_min = nxt_max, nxt_min
                    # transpose each 128-chunk into PSUM, then reduce over kp
                    for acc, ps_pool, coll, rop in (
                        (a_max, psA, c_max, Alu.max),
                        (a_min, psB, c_min, Alu.min),
                    ):
                        for half in range(M_PER // 2):
                            pst = fresh(ps_pool, [P, 2 * KT, P], FP16, "pst")
                            for c in range(2 * KT):
                                cc = half * 2 * KT + c
                                nc.tensor.transpose(
                                    pst[:, c], acc[:, cc * P : (cc + 1) * P], ident[:]
                                )
                            col0 = (mp * M_PER + half * 2) * KT
                            nc.vector.tensor_reduce(
                                out=coll[:, col0 : col0 + 2 * KT],
                                in_=pst[:],
                                axis=mybir.AxisListType.X,
                                op=rop,
                            )
                # assemble group: D = C_max + C_min, transpose back, add bias
                d = fresh(collp, [P, P * KT], FP16, "dsum")
                nc.vector.tensor_tensor(
                    out=d[:], in0=c_max[:], in1=c_min[:], op=Alu.add
                )
                stg = fresh(stgp, [P, N], FP32, "stg")
                for b in range(KT):
                    tbt = fresh(psA, [P, 2 * KT, P], FP16, "pst")
                    tb = tbt[:, 0]
                    nc.tensor.transpose(
                        tb, d.rearrange("p (m b) -> p m b", b=KT)[:, :, b], ident[:]
                    )
                    nc.vector.tensor_tensor(
                        out=stg[:, b * P : (b + 1) * P],
                        in0=tb,
                        in1=brep[:, b * P : (b + 1) * P],
                        op=Alu.add,
                    )
                nc.sync.dma_start(out_d[g], stg[:])
    if SPLIT_WAITS:
        _split_waits(nc, maxw=1)
    return nc


def _get_nc():
    if "nc" not in _compiled:
        _compiled["nc"] = _build()
    return _compiled["nc"]


def _prep_core(x16, w16t, b16, c):
    xs = x16[c * MC : (c + 1) * MC].astype(np.float32)  # fp16-rounded values, fp32 dtype
    xt = np.ascontiguousarray(xs.T.reshape(KT, P, MC))
    return {
        "xt": xt,
        "wt": w16t,
        "brep": np.ascontiguousarray(np.broadcast_to(b16[None, :], (P, N))),
        "ident": np.eye(P, dtype=np.float16),
    }


def kernel(x: np.ndarray, weight: np.ndarray, bias: np.ndarray) -> np.ndarray:
    from concourse.bass_utils import run_bass_kernel_spmd

    nc = _get_nc()
    x16 = np.ascontiguousarray(x).astype(np.float16)
    w16t = np.ascontiguousarray(
        np.ascontiguousarray(weight).astype(np.float16).T.reshape(KT, P, N)
    )
    b16 = np.ascontiguousarray(bias).astype(np.float16)

    in_maps = [_prep_core(x16, w16t, b16, c) for c in range(N_CORES)]
    if TRACE:
        try:
            import ntff_hook
            ntff_hook.install()
        except Exception:
            pass
    res = run_bass_kernel_spmd(
        nc, in_maps, core_ids=list(range(N_CORES)), trace=TRACE
    )
    LAST["res"] = res
    out = np.empty((M, N), dtype=np.float32)
    for c in range(N_CORES):
        out[c * MC : (c + 1) * MC] = res.results[c]["out"].reshape(MC, N)
    return out

